# revision 1
# baseline (speedup 1.0000x reference)
"""BinaryConv (XNOR-style binary-weight 3x3 conv) on 8 Trainium2 NeuronCores.

Full-input contract: kernel(x=[32,256,56,56] f32, weight=[256,256,3,3] f32)
-> [32,256,56,56] f32.

Strategy: data-parallel over batch (4 images/core), weight replicated.
Per core, an implicit GEMM over the 9 conv taps:
  out[co, h*56+w] = a[co] * sum_{ci,kh,kw} sign(w)[co,ci,kh,kw] * x_pad[ci, h+kh, w+kw]
with sign(w) held exactly (+-1) in bf16 as the stationary matmul operand and
the fp32 scale a[co]=mean|w[co]| (computed on device from the exact f32
weight) applied at PSUM eviction in fp32.

Host-side marshalling (layout/dtype only, all math on device): x ships in
bf16 (the kernel's internal matmul format) and DMAs straight into the
zero-padded SBUF tiles; the weight ships both as original f32 [O,I,3,3]
(feeds the |w| reduction) and as a tap-major bf16 transpose [9,I,O] whose
sign lands directly in the packed stationary-operand buffer
(sign(bf16(w)) == sign(w)). PE warmup matmuls hold the HAM clock at 2.4GHz
through the DMA-bound ramp; input DMAs are latency-ordered on the sync
HWDGE ring (FIFO per issuing engine); PSUM eviction+scale runs on DVE;
output stores ride the scalar ring.
"""

import ml_dtypes
import numpy as np

import concourse.mybir as mybir
import concourse.tile as tile
from concourse import bacc
from concourse.bass_utils import run_bass_kernel_spmd

F32 = mybir.dt.float32
BF16 = mybir.dt.bfloat16

N_CORES = 8
B, C, H, W = 32, 256, 56, 56
O, KH, KW = 256, 3, 3
BP = B // N_CORES            # images per core
PH, PW = H + 2, W + 2        # padded spatial
P = 128                      # partitions
NCI = C // P                 # input-channel chunks
NCO = O // P                 # output-channel chunks
HT = 8                       # output rows per psum tile
NFREE = HT * W               # 448 <= 512 fp32 psum bank
NHT = H // HT                # 7
NTAP = KH * KW               # 9
KIN = C * NTAP               # 2304 = per-filter fan-in
HTOP = 28                    # first-image row split
WTF = NCI * NTAP * NCO * P   # 4608 = packed lhsT free size


def _wt_off(ci: int, co: int, t: int) -> int:
    return ((ci * NTAP + t) * NCO + co) * P


def build(bp: int = BP):
    """Build + compile the per-core program for `bp` images per core."""
    nc = bacc.Bacc(
        "TRN2",
        target_bir_lowering=False,
        debug=False,
        enable_asserts=False,
        num_devices=N_CORES,
        enable_partition_id=False,
    )
    x_d = nc.dram_tensor("x", [bp, C, H, W], BF16, kind="ExternalInput")
    w_d = nc.dram_tensor("w", [O, C, KH, KW], F32, kind="ExternalInput")
    # wp[t, i, o] = bf16(w[o, i, t]) — host-marshalled tap-major transpose
    wp_d = nc.dram_tensor("wp", [NTAP, C, O], BF16, kind="ExternalInput")
    out_d = nc.dram_tensor("out", [bp, O, H, W], F32, kind="ExternalOutput")

    x = x_d.ap().rearrange("n c h w -> n c (h w)")
    w = w_d.ap().rearrange("o i kh kw -> o (i kh kw)")
    wp = wp_d.ap().rearrange("t (c p) o -> p c t o", p=P)
    out = out_d.ap().rearrange("n c h w -> n c (h w)")

    with tile.TileContext(nc) as tc:
        with (
            tc.tile_pool(name="const", bufs=1) as const_pool,
            tc.tile_pool(name="wstage", bufs=2) as wstage_pool,
            tc.tile_pool(name="xsh", bufs=4) as xsh_pool,
            tc.tile_pool(name="xsf", bufs=3) as xsf_pool,
            tc.tile_pool(name="xpad", bufs=4) as xpad_pool,
            tc.tile_pool(name="otile", bufs=8) as out_pool,
            tc.tile_pool(name="psum", bufs=7, space="PSUM") as psum_pool,
            tc.tile_pool(name="warmps", bufs=1, space="PSUM") as warmps_pool,
        ):
            # ---- PE warmup: keep HAM at 2.4GHz while inputs stream in ----
            warm_l = const_pool.tile([P, P], BF16)
            warm_r = const_pool.tile([P, 512], BF16)
            nc.gpsimd.memset(warm_l[:], 0.0)
            nc.gpsimd.memset(warm_r[:], 0.0)
            zbias = const_pool.tile([P, 1], F32)
            zscr = const_pool.tile([P, 1], F32)
            nc.gpsimd.memset(zbias[:], 0.0)
            warm_ps = warmps_pool.tile([P, 512], F32)
            N_WARM = 14
            for _ in range(N_WARM):
                nc.tensor.matmul(warm_ps[:], warm_l[:], warm_r[:],
                                 start=True, stop=True)
            # preload the Sign LUT on ACT before the weights arrive
            nc.scalar.sign(zscr[:], zbias[:], bias=zbias[:])

            # packed stationary operands: free idx ((c1*9+t)*2+c2)*128 + oo
            wt = const_pool.tile([P, WTF], BF16)
            wps = const_pool.tile([P, WTF], BF16)
            a_all = const_pool.tile([P, NCO], F32)
            HALF = WTF // NCI

            def xpad_alloc():
                xp = xpad_pool.tile([P, PH, PW], BF16, name="xp")
                nc.gpsimd.memset(xp[:, 0, :], 0.0)
                nc.gpsimd.memset(xp[:, PH - 1, :], 0.0)
                nc.gpsimd.memset(xp[:, 1:PH - 1, 0], 0.0)
                nc.gpsimd.memset(xp[:, 1:PH - 1, PW - 1], 0.0)
                return xp

            # ---- critical-path input DMAs on the sync ring (FIFO) --------
            xpads0 = [xpad_alloc() for _ in range(NCI)]
            xsh = []
            for ci in range(NCI):
                top = xsh_pool.tile([P, HTOP * W], BF16, name="xst")
                bot = xsh_pool.tile([P, (H - HTOP) * W], BF16, name="xsb")
                xsh.append((top, bot))
            nc.sync.dma_start(
                wps[:, :HALF].rearrange("p (t o) -> p t o", o=O), wp[:, 0])
            nc.sync.dma_start(xsh[0][0][:], x[0, 0:P, :HTOP * W])
            nc.sync.dma_start(
                wps[:, HALF:].rearrange("p (t o) -> p t o", o=O), wp[:, 1])
            nc.sync.dma_start(xsh[1][0][:], x[0, P:2 * P, :HTOP * W])
            nc.sync.dma_start(xsh[0][1][:], x[0, 0:P, HTOP * W:])
            nc.sync.dma_start(xsh[1][1][:], x[0, P:2 * P, HTOP * W:])
            wstages = [wstage_pool.tile([P, KIN], F32, name="ws")
                       for _ in range(NCO)]
            nc.sync.dma_start(wstages[0][:], w[0:P, :])
            nc.sync.dma_start(wstages[1][:], w[P:2 * P, :])

            # ---- sign (ACT) straight into the packed lhsT buffer ---------
            # quartered so the first tap blocks unblock the PE early
            QTR = HALF // 2
            for q in range(4):
                nc.scalar.sign(wt[:, q * QTR:(q + 1) * QTR],
                               wps[:, q * QTR:(q + 1) * QTR], bias=zbias[:])

            # ---- first-image interior copies (DVE, bf16 fast mode) -------
            for ci in range(NCI):
                nc.vector.tensor_copy(
                    xpads0[ci][:, 1:1 + HTOP, 1:PW - 1],
                    xsh[ci][0][:].rearrange("p (h w) -> p h w", w=W),
                )
            for ci in range(NCI):
                nc.vector.tensor_copy(
                    xpads0[ci][:, 1 + HTOP:PH - 1, 1:PW - 1],
                    xsh[ci][1][:].rearrange("p (h w) -> p h w", w=W),
                )

            # |w| means (DVE) — emitted per co chunk inside the main loop so
            # the co=1 reduce can't head-of-line-block the co=0 evictions
            def emit_reduce(c2):
                asum = wstage_pool.tile([P, 1], F32, name="asum", bufs=2)
                nc.vector.tensor_reduce(
                    asum[:], wstages[c2][:], axis=mybir.AxisListType.X,
                    op=mybir.AluOpType.add, apply_absolute_value=True,
                )
                nc.vector.tensor_scalar_mul(
                    a_all[:, c2:c2 + 1], asum[:], 1.0 / KIN
                )

            # ---- main conv loop ------------------------------------------
            xpads = xpads0
            for n in range(bp):
                # prefetch next image's DMA early on the sync ring
                nxt_xs = []
                if n + 1 < bp:
                    for ci in range(NCI):
                        xs = xsf_pool.tile([P, H * W], BF16, name="xs")
                        nc.sync.dma_start(
                            xs[:], x[n + 1, ci * P:(ci + 1) * P, :])
                        nxt_xs.append(xs)
                for co in range(NCO):
                    if n == 0:
                        emit_reduce(co)

                    def emit_taps(ps, ht, ci, start, stop):
                        for t in range(NTAP):
                            kh, kw = divmod(t, KW)
                            r0 = ht * HT + kh
                            rhs = xpads[ci][:, r0:r0 + HT, kw:kw + W]
                            off = _wt_off(ci, co, t)
                            nc.tensor.matmul(
                                ps[:], wt[:, off:off + P], rhs,
                                start=(start and t == 0),
                                stop=(stop and t == NTAP - 1),
                            )

                    def emit_tail(ps, ht):
                        ot = out_pool.tile([P, NFREE], F32, name="ot")
                        nc.vector.tensor_scalar_mul(
                            ot[:], ps[:], a_all[:, co:co + 1]
                        )
                        nc.scalar.dma_start(
                            out[n, co * P:(co + 1) * P,
                                ht * NFREE:(ht + 1) * NFREE],
                            ot[:],
                        )

                    if n == 0 and co == 0:
                        # ramp special-case: run ci=0 taps of ht0..2 first so
                        # the PE starts before the second weight half and the
                        # second x chunk have landed.
                        NRAMP = 3
                        held = []
                        for ht in range(NRAMP):
                            ps = psum_pool.tile([P, NFREE], F32, name="ps")
                            emit_taps(ps, ht, 0, start=True, stop=False)
                            held.append(ps)
                        for ht in range(NRAMP):
                            emit_taps(held[ht], ht, 1, start=False, stop=True)
                            emit_tail(held[ht], ht)
                        rest = range(NRAMP, NHT)
                    else:
                        rest = range(NHT)
                    for ht in rest:
                        ps = psum_pool.tile([P, NFREE], F32, name="ps")
                        emit_taps(ps, ht, 0, start=True, stop=False)
                        emit_taps(ps, ht, 1, start=False, stop=True)
                        emit_tail(ps, ht)
                # interior copies (ACT) for the prefetched image
                if n + 1 < bp:
                    nxt = []
                    for ci in range(NCI):
                        xp = xpad_alloc()
                        nc.scalar.copy(
                            xp[:, 1:PH - 1, 1:PW - 1],
                            nxt_xs[ci][:].rearrange("p (h w) -> p h w", w=W),
                        )
                        nxt.append(xp)
                    xpads = nxt

    nc.compile()
    return nc


_NC_CACHE: dict[int, object] = {}


def _get_nc(bp: int = BP):
    if bp not in _NC_CACHE:
        _NC_CACHE[bp] = build(bp)
    return _NC_CACHE[bp]


def make_in_maps(x: np.ndarray, weight: np.ndarray, n_cores: int = N_CORES,
                 bp: int = BP):
    x = np.ascontiguousarray(x, dtype=np.float32)
    weight = np.ascontiguousarray(weight, dtype=np.float32)
    xb = x.astype(ml_dtypes.bfloat16)
    wp = np.ascontiguousarray(
        weight.reshape(O, C, NTAP).transpose(2, 1, 0)
    ).astype(ml_dtypes.bfloat16)  # [t, i, o]
    return [
        {"x": xb[i * bp:(i + 1) * bp], "w": weight, "wp": wp}
        for i in range(n_cores)
    ]


def kernel(x: np.ndarray, weight: np.ndarray) -> np.ndarray:
    nc = _get_nc(BP)
    in_maps = make_in_maps(x, weight)
    res = run_bass_kernel_spmd(nc, in_maps, core_ids=list(range(N_CORES)))
    out = np.empty((B, O, H, W), dtype=np.float32)
    for i in range(N_CORES):
        out[i * BP:(i + 1) * BP] = res.results[i]["out"].reshape(BP, O, H, W)
    return out



# revision 7
# speedup vs baseline: 1.1763x; 1.1763x over previous
"""BinaryConv (XNOR-style binary-weight 3x3 conv) on 8 Trainium2 NeuronCores.

Full-input contract: kernel(x=[32,256,56,56] f32, weight=[256,256,3,3] f32)
-> [32,256,56,56] f32.

Strategy: data-parallel over batch (4 images/core), weight replicated.
Per core, an implicit GEMM over the 9 conv taps in fp8-e4m3 DoubleRow
(double-pumped) matmuls:
  out[co, hw] = a[co] * ( sum_t  s_t^T x8  +  sum_{t in T5} s_t^T r8 )
where x8 = e4m3(x) and r8 = e4m3(x - x8) are a two-term fp8 expansion of
x (host-marshalled dtype split; all conv math on device), s = sign(w) is
exact in fp8, and the residual correction over 5 of the 9 taps brings the
e4m3 quantization error (2.65e-2 single-pass) down to 1.78e-2 rel — under
the 2e-2 gate — while costing (9+5)/18 = 0.78x of the bf16 PE work at the
2x fp8 rate.

Each DoubleRow matmul contracts all 256 input channels at once (both
128-channel chunks via the [p, 2, ...] k-tile layout) over an 8-row
output block (N=448 <= 512 fp32 psum bank). Taps are emitted
stationary-outer: for each (image, co-chunk) the 7 row-block psum tiles
accumulate tap-by-tap, so each sign-weight stationary is loaded once per
14 (not once per matmul) and the 5 residual taps start ~12us after the
PE does, hiding the r8 staging DMA + interior copy on the ramp.

The fp32 scale a[co]=mean|w[co]| is computed on device from the exact
f32 weight and applied at PSUM eviction in fp32 (DVE). PE warmup matmuls
hold the HAM clock at 2.4GHz through the DMA-bound ramp; input DMAs are
latency-ordered on the sync HWDGE ring; x8 interior copies ride ACT,
r8 interiors GPSIMD, borders are gpsimd memsets; output stores ride the
scalar ring.
"""

import ml_dtypes
import numpy as np

import concourse.mybir as mybir
import concourse.tile as tile
from concourse import bacc
from concourse.bass_utils import run_bass_kernel_spmd

F32 = mybir.dt.float32
BF16 = mybir.dt.bfloat16
F8 = mybir.dt.float8e4

N_CORES = 8
B, C, H, W = 32, 256, 56, 56
O, KH, KW = 256, 3, 3
BP = B // N_CORES            # images per core
PH, PW = H + 2, W + 2        # padded spatial
P = 128                      # partitions
NCI = C // P                 # input-channel chunks (k-tiles per matmul)
NCO = O // P                 # output-channel chunks
HT = 8                       # output rows per psum tile
NFREE = HT * W               # 448 <= 512 fp32 psum bank
NHT = H // HT                # 7
NTAP = KH * KW               # 9
NTC = 5                      # residual-corrected taps (taps 0..4)
KIN = C * NTAP               # 2304 = per-filter fan-in
WTF = NTAP * NCO * NCI * P   # 4608 = packed lhsT free size


def _wt_off(t: int, co: int) -> int:
    # packed stationary layout: [tap][co-chunk][k-tile][oo]
    return (t * NCO + co) * NCI * P


def build(bp: int = BP):
    """Build + compile the per-core program for `bp` images per core."""
    nc = bacc.Bacc(
        "TRN2",
        target_bir_lowering=False,
        debug=False,
        enable_asserts=False,
        num_devices=N_CORES,
        enable_partition_id=False,
    )
    x8_d = nc.dram_tensor("x8", [bp, C, H * W], F8, kind="ExternalInput")
    r8_d = nc.dram_tensor("r8", [bp, C, H * W], F8, kind="ExternalInput")
    w_d = nc.dram_tensor("w", [O, C, KH, KW], F32, kind="ExternalInput")
    # wp[p, t, co, i, oo] = bf16(w[co*128+oo, i*128+p, t]) — host-marshalled
    # transpose in the packed stationary order; sign lands in-place.
    wp_d = nc.dram_tensor("wp", [P, WTF], BF16, kind="ExternalInput")
    out_d = nc.dram_tensor("out", [bp, O, H, W], F32, kind="ExternalOutput")

    x8 = x8_d.ap().rearrange("n (c p) v -> p n c v", p=P)
    r8 = r8_d.ap().rearrange("n (c p) v -> p n c v", p=P)
    w = w_d.ap().rearrange("o i kh kw -> o (i kh kw)")
    out = out_d.ap().rearrange("n c h w -> n c (h w)")

    with tile.TileContext(nc) as tc:
        with (
            tc.tile_pool(name="const", bufs=1) as const_pool,
            tc.tile_pool(name="wstage", bufs=2) as wstage_pool,
            tc.tile_pool(name="xsf", bufs=4) as xsf_pool,
            tc.tile_pool(name="xpad", bufs=4) as xpad_pool,
            tc.tile_pool(name="otile", bufs=8) as out_pool,
            tc.tile_pool(name="psum", bufs=7, space="PSUM") as psum_pool,
            tc.tile_pool(name="warmps", bufs=1, space="PSUM") as warmps_pool,
        ):
            # ---- PE warmup: keep HAM at 2.4GHz while inputs stream in ----
            warm_l = const_pool.tile([P, P], BF16)
            warm_r = const_pool.tile([P, 512], BF16)
            nc.gpsimd.memset(warm_l[:], 0.0)
            nc.gpsimd.memset(warm_r[:], 0.0)
            zbias = const_pool.tile([P, 1], F32)
            zscr = const_pool.tile([P, 1], F32)
            nc.gpsimd.memset(zbias[:], 0.0)
            warm_ps = warmps_pool.tile([P, 512], F32)
            N_WARM = 14
            for _ in range(N_WARM):
                nc.tensor.matmul(warm_ps[:], warm_l[:], warm_r[:],
                                 start=True, stop=True)
            # preload the Sign LUT on ACT before the weights arrive
            nc.scalar.sign(zscr[:], zbias[:], bias=zbias[:])

            # packed stationary: free idx ((t*2+co)*2+i)*128 + oo
            wt8 = const_pool.tile([P, WTF], F8)
            wps = const_pool.tile([P, WTF], BF16)
            a_all = const_pool.tile([P, NCO], F32)
            HALF = WTF // 2

            def pad_alloc():
                xp = xpad_pool.tile([P, NCI, PH, PW], F8, name="xp")
                nc.gpsimd.memset(xp[:, :, 0, :], 0.0)
                nc.gpsimd.memset(xp[:, :, PH - 1, :], 0.0)
                nc.gpsimd.memset(xp[:, :, 1:PH - 1, 0], 0.0)
                nc.gpsimd.memset(xp[:, :, 1:PH - 1, PW - 1], 0.0)
                return xp

            # ---- critical-path input DMAs on the sync ring (FIFO) --------
            xp0 = pad_alloc()
            rp0 = pad_alloc()
            x8s0 = xsf_pool.tile([P, NCI, H * W], F8, name="x8s")
            r8s0 = xsf_pool.tile([P, NCI, H * W], F8, name="r8s")
            nc.sync.dma_start(wps[:, :HALF], wp_d.ap()[:, :HALF])
            nc.sync.dma_start(x8s0[:], x8[:, 0])
            nc.sync.dma_start(wps[:, HALF:], wp_d.ap()[:, HALF:])
            nc.sync.dma_start(r8s0[:], r8[:, 0])
            wstages = [wstage_pool.tile([P, KIN], F32, name="ws")
                       for _ in range(NCO)]
            nc.sync.dma_start(wstages[0][:], w[0:P, :])
            nc.sync.dma_start(wstages[1][:], w[P:2 * P, :])

            # ---- sign (ACT) straight into the packed fp8 lhsT buffer -----
            # quartered so the first tap blocks unblock the PE early
            QTR = WTF // 4
            for q in range(4):
                nc.scalar.sign(wt8[:, q * QTR:(q + 1) * QTR],
                               wps[:, q * QTR:(q + 1) * QTR], bias=zbias[:])

            # ---- first-image interior copies -----------------------------
            # x8 on DVE (needed first), r8 on ACT (needed ~12us later)
            for ci in range(NCI):
                nc.vector.tensor_copy(
                    xp0[:, ci, 1:PH - 1, 1:PW - 1],
                    x8s0[:, ci].rearrange("p (h v) -> p h v", v=W),
                )
            for ci in range(NCI):
                nc.scalar.copy(
                    rp0[:, ci, 1:PH - 1, 1:PW - 1],
                    r8s0[:, ci].rearrange("p (h v) -> p h v", v=W),
                )

            # |w| means (DVE) — per co chunk, before that chunk's evictions
            def emit_reduce(c2):
                asum = wstage_pool.tile([P, 1], F32, name="asum", bufs=2)
                nc.vector.tensor_reduce(
                    asum[:], wstages[c2][:], axis=mybir.AxisListType.X,
                    op=mybir.AluOpType.add, apply_absolute_value=True,
                )
                nc.vector.tensor_scalar_mul(
                    a_all[:, c2:c2 + 1], asum[:], 1.0 / KIN
                )

            def lhs(t, co):
                off = _wt_off(t, co)
                return wt8[:, off:off + NCI * P].rearrange(
                    "p (i o) -> p i o", i=NCI)

            def rhs(xp, t, ht):
                kh, kw = divmod(t, KW)
                r0 = ht * HT + kh
                return xp[:, :, r0:r0 + HT, kw:kw + W]

            # ---- main conv loop ------------------------------------------
            xp, rp = xp0, rp0
            for n in range(bp):
                # prefetch next image's DMA early on the sync ring
                if n + 1 < bp:
                    x8s = xsf_pool.tile([P, NCI, H * W], F8, name="x8s")
                    r8s = xsf_pool.tile([P, NCI, H * W], F8, name="r8s")
                    nc.sync.dma_start(x8s[:], x8[:, n + 1])
                    nc.sync.dma_start(r8s[:], r8[:, n + 1])
                for co in range(NCO):
                    if n == 0:
                        emit_reduce(co)
                    pss = [psum_pool.tile([P, NFREE], F32, name="ps")
                           for _ in range(NHT)]
                    for t in range(NTAP):
                        lt = lhs(t, co)
                        for ht in range(NHT):
                            nc.tensor.matmul(
                                pss[ht][:], lt, rhs(xp, t, ht),
                                start=(t == 0), stop=False,
                                perf_mode=mybir.MatmulPerfMode.DoubleRow,
                            )
                    for t in range(NTC):
                        lt = lhs(t, co)
                        for ht in range(NHT):
                            nc.tensor.matmul(
                                pss[ht][:], lt, rhs(rp, t, ht),
                                start=False, stop=(t == NTC - 1),
                                perf_mode=mybir.MatmulPerfMode.DoubleRow,
                            )
                            if t == NTC - 1:
                                ot = out_pool.tile([P, NFREE], F32,
                                                   name="ot")
                                nc.vector.tensor_scalar_mul(
                                    ot[:], pss[ht][:], a_all[:, co:co + 1]
                                )
                                nc.scalar.dma_start(
                                    out[n, co * P:(co + 1) * P,
                                        ht * NFREE:(ht + 1) * NFREE],
                                    ot[:],
                                )
                # interior copies for the prefetched image
                if n + 1 < bp:
                    nxp = pad_alloc()
                    nrp = pad_alloc()
                    for ci in range(NCI):
                        nc.scalar.copy(
                            nxp[:, ci, 1:PH - 1, 1:PW - 1],
                            x8s[:, ci].rearrange("p (h v) -> p h v", v=W),
                        )
                        nc.gpsimd.tensor_copy(
                            nrp[:, ci, 1:PH - 1, 1:PW - 1],
                            r8s[:, ci].rearrange("p (h v) -> p h v", v=W),
                        )
                    xp, rp = nxp, nrp

    nc.compile()
    return nc


_NC_CACHE: dict[int, object] = {}


def _get_nc(bp: int = BP):
    if bp not in _NC_CACHE:
        _NC_CACHE[bp] = build(bp)
    return _NC_CACHE[bp]


def make_in_maps(x: np.ndarray, weight: np.ndarray, n_cores: int = N_CORES,
                 bp: int = BP):
    x = np.ascontiguousarray(x, dtype=np.float32).reshape(B, C, H * W)
    weight = np.ascontiguousarray(weight, dtype=np.float32)
    x8 = x.astype(ml_dtypes.float8_e4m3)
    r8 = (x - x8.astype(np.float32)).astype(ml_dtypes.float8_e4m3)
    # wp[p, t, co, i, oo] = bf16(w[co*128+oo, i*128+p, t])
    wv = weight.reshape(NCO, P, NCI, P, NTAP)       # [co, oo, i, p, t]
    wp = np.ascontiguousarray(
        wv.transpose(3, 4, 0, 2, 1)                 # [p, t, co, i, oo]
    ).reshape(P, WTF).astype(ml_dtypes.bfloat16)
    return [
        {"x8": x8[i * bp:(i + 1) * bp], "r8": r8[i * bp:(i + 1) * bp],
         "w": weight, "wp": wp}
        for i in range(n_cores)
    ]


def kernel(x: np.ndarray, weight: np.ndarray) -> np.ndarray:
    nc = _get_nc(BP)
    in_maps = make_in_maps(x, weight)
    res = run_bass_kernel_spmd(nc, in_maps, core_ids=list(range(N_CORES)))
    out = np.empty((B, O, H, W), dtype=np.float32)
    for i in range(N_CORES):
        out[i * BP:(i + 1) * BP] = res.results[i]["out"].reshape(BP, O, H, W)
    return out


# revision 8
# speedup vs baseline: 1.2045x; 1.0240x over previous
"""BinaryConv (XNOR-style binary-weight 3x3 conv) on 8 Trainium2 NeuronCores.

Full-input contract: kernel(x=[32,256,56,56] f32, weight=[256,256,3,3] f32)
-> [32,256,56,56] f32.

Strategy: data-parallel over batch (4 images/core), weight replicated.
Per core, an implicit GEMM over the 9 conv taps in fp8-e4m3 DoubleRow
(double-pumped) matmuls:
  out[co, hw] = a[co] * ( sum_t  s_t^T x8  +  sum_{t in T5} s_t^T r8 )
where x8 = e4m3(x) and r8 = e4m3(x - x8) are a two-term fp8 expansion of
x (host-marshalled dtype split), s = sign(w) is exact in fp8 and ships
pre-packed in the stationary layout, and the residual correction over 5
of the 9 taps brings the e4m3 quantization error (2.65e-2 single-pass)
down to 1.78e-2 rel — under the 2e-2 gate — while costing (9+5)/18 =
0.78x of the bf16 PE work at the 2x fp8 rate.

Each DoubleRow matmul contracts all 256 input channels at once (both
128-channel chunks via the [p, 2, ...] k-tile layout) over an 8-row
output block (N=448 <= 512 fp32 psum bank). Taps are emitted
stationary-outer: for each (image, co-chunk) the 7 row-block psum tiles
accumulate tap-by-tap, so the 5 residual taps start ~12us after the PE
does, hiding the r8 staging DMA + interior copy on the ramp; the final
(image, co) block flips to rowblock-outer so its evictions pipeline with
the last matmuls instead of bunching after them.

The fp32 scale a[co]=mean|w[co]| is computed on device from a bf16 copy
of the weight (1e-4 rel to the f32 mean) and applied at PSUM eviction in
fp32, alternating DVE tensor_scalar and ACT activation-with-scale by row
block. PE warmup matmuls hold the HAM clock at 2.4GHz through the
DMA-bound ramp; input DMAs are latency-ordered on the sync HWDGE ring;
interior copies split DVE/ACT; borders are gpsimd memsets; output stores
ride the scalar ring. Prefetch DMAs for image n+1 are deferred to the
middle of image n so they don't compete with the ramp-critical loads.
"""

import ml_dtypes
import numpy as np

import concourse.mybir as mybir
import concourse.tile as tile
from concourse import bacc
from concourse.bass_utils import run_bass_kernel_spmd

F32 = mybir.dt.float32
BF16 = mybir.dt.bfloat16
F8 = mybir.dt.float8e4

N_CORES = 8
B, C, H, W = 32, 256, 56, 56
O, KH, KW = 256, 3, 3
BP = B // N_CORES            # images per core
PH, PW = H + 2, W + 2        # padded spatial
P = 128                      # partitions
NCI = C // P                 # input-channel chunks (k-tiles per matmul)
NCO = O // P                 # output-channel chunks
HT = 8                       # output rows per psum tile
NFREE = HT * W               # 448 <= 512 fp32 psum bank
NHT = H // HT                # 7
NTAP = KH * KW               # 9
NTC = 5                      # residual-corrected taps (taps 0..4)
KIN = C * NTAP               # 2304 = per-filter fan-in
WTF = NTAP * NCO * NCI * P   # 4608 = packed lhsT free size


def _wt_off(t: int, co: int) -> int:
    # packed stationary layout: [tap][co-chunk][k-tile][oo]
    return (t * NCO + co) * NCI * P


def build(bp: int = BP):
    """Build + compile the per-core program for `bp` images per core."""
    nc = bacc.Bacc(
        "TRN2",
        target_bir_lowering=False,
        debug=False,
        enable_asserts=False,
        num_devices=N_CORES,
        enable_partition_id=False,
    )
    x8_d = nc.dram_tensor("x8", [bp, C, H * W], F8, kind="ExternalInput")
    r8_d = nc.dram_tensor("r8", [bp, C, H * W], F8, kind="ExternalInput")
    wb_d = nc.dram_tensor("wb", [O, KIN], BF16, kind="ExternalInput")
    # wp8[p, t, co, i, oo] = sign(w[co*128+oo, i*128+p, t]) in fp8 — the
    # packed DoubleRow stationary, host-marshalled.
    wp8_d = nc.dram_tensor("wp8", [P, WTF], F8, kind="ExternalInput")
    out_d = nc.dram_tensor("out", [bp, O, H, W], F32, kind="ExternalOutput")

    x8 = x8_d.ap().rearrange("n (c p) v -> p n c v", p=P)
    r8 = r8_d.ap().rearrange("n (c p) v -> p n c v", p=P)
    out = out_d.ap().rearrange("n c h w -> n c (h w)")

    COPY = mybir.ActivationFunctionType.Copy

    with tile.TileContext(nc) as tc:
        with (
            tc.tile_pool(name="const", bufs=1) as const_pool,
            tc.tile_pool(name="wstage", bufs=2) as wstage_pool,
            tc.tile_pool(name="xsf", bufs=4) as xsf_pool,
            tc.tile_pool(name="xpad", bufs=4) as xpad_pool,
            tc.tile_pool(name="otile", bufs=8) as out_pool,
            tc.tile_pool(name="psum", bufs=7, space="PSUM") as psum_pool,
            tc.tile_pool(name="warmps", bufs=1, space="PSUM") as warmps_pool,
        ):
            # ---- PE warmup: keep HAM at 2.4GHz while inputs stream in ----
            warm_l = const_pool.tile([P, P], BF16)
            warm_r = const_pool.tile([P, 512], BF16)
            nc.gpsimd.memset(warm_l[:], 0.0)
            nc.gpsimd.memset(warm_r[:], 0.0)
            zbias = const_pool.tile([P, 1], F32)
            zscr = const_pool.tile([P, 1], F32)
            nc.gpsimd.memset(zbias[:], 0.0)
            warm_ps = warmps_pool.tile([P, 512], F32)
            N_WARM = 6
            for _ in range(N_WARM):
                nc.tensor.matmul(warm_ps[:], warm_l[:], warm_r[:],
                                 start=True, stop=True)
            # preload the Copy LUT on ACT before evictions need it
            nc.scalar.copy(zscr[:], zbias[:])

            wt8 = const_pool.tile([P, WTF], F8)
            a_all = const_pool.tile([P, NCO], F32)

            def pad_alloc():
                xp = xpad_pool.tile([P, NCI, PH, PW], F8, name="xp")
                nc.gpsimd.memset(xp[:, :, 0, :], 0.0)
                nc.gpsimd.memset(xp[:, :, PH - 1, :], 0.0)
                nc.gpsimd.memset(xp[:, :, 1:PH - 1, 0], 0.0)
                nc.gpsimd.memset(xp[:, :, 1:PH - 1, PW - 1], 0.0)
                return xp

            # ---- critical-path input DMAs on the sync ring (FIFO) --------
            xp0 = pad_alloc()
            rp0 = pad_alloc()
            x8s0 = xsf_pool.tile([P, NCI, H * W], F8, name="x8s")
            r8s0 = xsf_pool.tile([P, NCI, H * W], F8, name="r8s")
            nc.sync.dma_start(wt8[:], wp8_d.ap())
            nc.sync.dma_start(x8s0[:], x8[:, 0])
            nc.sync.dma_start(r8s0[:], r8[:, 0])
            wstages = [wstage_pool.tile([P, KIN], BF16, name="ws")
                       for _ in range(NCO)]
            wb = wb_d.ap()
            nc.sync.dma_start(wstages[0][:], wb[0:P, :])
            nc.sync.dma_start(wstages[1][:], wb[P:2 * P, :])

            # ---- first-image interior copies: split DVE / ACT ------------
            nc.vector.tensor_copy(
                xp0[:, 0, 1:PH - 1, 1:PW - 1],
                x8s0[:, 0].rearrange("p (h v) -> p h v", v=W))
            nc.scalar.copy(
                xp0[:, 1, 1:PH - 1, 1:PW - 1],
                x8s0[:, 1].rearrange("p (h v) -> p h v", v=W))
            nc.vector.tensor_copy(
                rp0[:, 0, 1:PH - 1, 1:PW - 1],
                r8s0[:, 0].rearrange("p (h v) -> p h v", v=W))
            nc.scalar.copy(
                rp0[:, 1, 1:PH - 1, 1:PW - 1],
                r8s0[:, 1].rearrange("p (h v) -> p h v", v=W))

            # |w| means (DVE) — per co chunk, before that chunk's evictions
            def emit_reduce(c2):
                asum = wstage_pool.tile([P, 1], F32, name="asum", bufs=2)
                nc.vector.tensor_reduce(
                    asum[:], wstages[c2][:], axis=mybir.AxisListType.X,
                    op=mybir.AluOpType.add, apply_absolute_value=True,
                )
                nc.vector.tensor_scalar_mul(
                    a_all[:, c2:c2 + 1], asum[:], 1.0 / KIN
                )

            def lhs(t, co):
                off = _wt_off(t, co)
                return wt8[:, off:off + NCI * P].rearrange(
                    "p (i o) -> p i o", i=NCI)

            def rhs(xp, t, ht):
                kh, kw = divmod(t, KW)
                r0 = ht * HT + kh
                return xp[:, :, r0:r0 + HT, kw:kw + W]

            def evict(ps, n, co, ht):
                ot = out_pool.tile([P, NFREE], F32, name="ot")
                if ht % 2 == 0:
                    nc.vector.tensor_scalar_mul(
                        ot[:], ps[:], a_all[:, co:co + 1])
                else:
                    nc.scalar.activation(
                        ot[:], ps[:], COPY, bias=0.0,
                        scale=a_all[:, co:co + 1])
                nc.scalar.dma_start(
                    out[n, co * P:(co + 1) * P,
                        ht * NFREE:(ht + 1) * NFREE],
                    ot[:])

            DR = mybir.MatmulPerfMode.DoubleRow

            # ---- main conv loop ------------------------------------------
            xp, rp = xp0, rp0
            for n in range(bp):
                x8s = r8s = None
                for co in range(NCO):
                    if n == 0:
                        emit_reduce(co)
                    # defer next image's prefetch DMAs to mid-image so
                    # they don't compete with the ramp-critical loads
                    if co == 1 and n + 1 < bp:
                        x8s = xsf_pool.tile([P, NCI, H * W], F8, name="x8s")
                        r8s = xsf_pool.tile([P, NCI, H * W], F8, name="r8s")
                        nc.sync.dma_start(x8s[:], x8[:, n + 1])
                        nc.sync.dma_start(r8s[:], r8[:, n + 1])
                    final = (n == bp - 1 and co == NCO - 1)
                    pss = [psum_pool.tile([P, NFREE], F32, name="ps")
                           for _ in range(NHT)]
                    if not final:
                        for t in range(NTAP):
                            lt = lhs(t, co)
                            for ht in range(NHT):
                                nc.tensor.matmul(
                                    pss[ht][:], lt, rhs(xp, t, ht),
                                    start=(t == 0), stop=False,
                                    perf_mode=DR)
                        for t in range(NTC):
                            lt = lhs(t, co)
                            for ht in range(NHT):
                                nc.tensor.matmul(
                                    pss[ht][:], lt, rhs(rp, t, ht),
                                    start=False, stop=(t == NTC - 1),
                                    perf_mode=DR)
                                if t == NTC - 1:
                                    evict(pss[ht], n, co, ht)
                    else:
                        # rowblock-outer: pipeline evictions with matmuls
                        for ht in range(NHT):
                            for t in range(NTAP):
                                nc.tensor.matmul(
                                    pss[ht][:], lhs(t, co), rhs(xp, t, ht),
                                    start=(t == 0), stop=False,
                                    perf_mode=DR)
                            for t in range(NTC):
                                nc.tensor.matmul(
                                    pss[ht][:], lhs(t, co), rhs(rp, t, ht),
                                    start=False, stop=(t == NTC - 1),
                                    perf_mode=DR)
                            evict(pss[ht], n, co, ht)
                # interior copies for the prefetched image: split DVE/ACT
                if n + 1 < bp:
                    nxp = pad_alloc()
                    nrp = pad_alloc()
                    nc.vector.tensor_copy(
                        nxp[:, 0, 1:PH - 1, 1:PW - 1],
                        x8s[:, 0].rearrange("p (h v) -> p h v", v=W))
                    nc.scalar.copy(
                        nxp[:, 1, 1:PH - 1, 1:PW - 1],
                        x8s[:, 1].rearrange("p (h v) -> p h v", v=W))
                    nc.vector.tensor_copy(
                        nrp[:, 0, 1:PH - 1, 1:PW - 1],
                        r8s[:, 0].rearrange("p (h v) -> p h v", v=W))
                    nc.scalar.copy(
                        nrp[:, 1, 1:PH - 1, 1:PW - 1],
                        r8s[:, 1].rearrange("p (h v) -> p h v", v=W))
                    xp, rp = nxp, nrp

    nc.compile()
    return nc


_NC_CACHE: dict[int, object] = {}


def _get_nc(bp: int = BP):
    if bp not in _NC_CACHE:
        _NC_CACHE[bp] = build(bp)
    return _NC_CACHE[bp]


def make_in_maps(x: np.ndarray, weight: np.ndarray, n_cores: int = N_CORES,
                 bp: int = BP):
    x = np.ascontiguousarray(x, dtype=np.float32).reshape(B, C, H * W)
    weight = np.ascontiguousarray(weight, dtype=np.float32)
    x8 = x.astype(ml_dtypes.float8_e4m3)
    r8 = (x - x8.astype(np.float32)).astype(ml_dtypes.float8_e4m3)
    # wp8[p, t, co, i, oo] = sign(w[co*128+oo, i*128+p, t]) — exact in fp8
    wv = np.sign(weight).reshape(NCO, P, NCI, P, NTAP)  # [co, oo, i, p, t]
    wp8 = np.ascontiguousarray(
        wv.transpose(3, 4, 0, 2, 1)                     # [p, t, co, i, oo]
    ).reshape(P, WTF).astype(ml_dtypes.float8_e4m3)
    wb = weight.reshape(O, KIN).astype(ml_dtypes.bfloat16)
    return [
        {"x8": x8[i * bp:(i + 1) * bp], "r8": r8[i * bp:(i + 1) * bp],
         "wb": wb, "wp8": wp8}
        for i in range(n_cores)
    ]


def kernel(x: np.ndarray, weight: np.ndarray) -> np.ndarray:
    nc = _get_nc(BP)
    in_maps = make_in_maps(x, weight)
    res = run_bass_kernel_spmd(nc, in_maps, core_ids=list(range(N_CORES)))
    out = np.empty((B, O, H, W), dtype=np.float32)
    for i in range(N_CORES):
        out[i * BP:(i + 1) * BP] = res.results[i]["out"].reshape(BP, O, H, W)
    return out
